# revision 1
# baseline (speedup 1.0000x reference)
"""v4: interleaved qkv slices + bf16 x. Causal self-attention (B=4, T=2048, C=1024, H=16) on 8 TRN2 NeuronCores.

Sharding (tensor-parallel over batch x head-group): core c handles batch c//2
and heads [8*(c%2), 8*(c%2)+8). Host sums the two head-group partials per
batch and adds the bias.

v3 on top of v2 (fp8-DoubleRow PV with residual correction):
  - Interleaved schedule: the QKV projection is computed per 512-token chunk
    tj, immediately followed by the attention block j=tj (causality only
    needs K/V chunks <= tj). The PE's projection work for chunk j+1 then
    overlaps the ScalarE softmax of block j instead of serializing.
  - qkT stored bf16 (same PE rate, halves SBUF) so phase-1 and phase-2
    pools can coexist.
  - ones-slots of the v stacks initialized once outside the rep loop;
  - combine DMAs (corr/dn/ytmp) dispatched from the ACT hardware DGE queue
    to unload the SP queue; output tiles merged to [128,1024] per DMA.
  - per pair the odd head is processed first so its SBUF partition-remap
    DMA overlaps the even head's combine.
"""
from contextlib import ExitStack

import numpy as np
import concourse.bass as bass
import concourse.mybir as mybir
import concourse.tile as tile
from concourse import bacc

F32 = mybir.dt.float32
F32R = mybir.dt.float32r
BF16 = mybir.dt.bfloat16
F8 = mybir.dt.float8e4
EXP = mybir.ActivationFunctionType.Exp
DR = mybir.MatmulPerfMode.DoubleRow
MUL = mybir.AluOpType.mult
ADD = mybir.AluOpType.add

T = 2048          # tokens
C = 1024          # channels
NH = 8            # local heads
HD = 64           # head dim
CL = NH * HD      # local channels (512)
TJ = T // 512     # 4 q-chunks of 512
KC = T // 128     # 16 k-chunks of 128
SCALE = HD ** -0.5
BIAS = -3.0       # exp bias for fp8 att storage (cancels in normalization)


def build_nc(loop_reps: int | None = None):
    nc = bacc.Bacc("TRN2", target_bir_lowering=False, debug=False)
    xT = nc.declare_dram_parameter("xT", [C, T], BF16, isOutput=False)
    wqk = nc.declare_dram_parameter("wqk", [C, 2 * CL], BF16, isOutput=False)
    wv = nc.declare_dram_parameter("wv", [C, CL], BF16, isOutput=False)
    wp = nc.declare_dram_parameter("wp", [4, 128, C], F32R, isOutput=False)
    idn = nc.declare_dram_parameter("idn", [128, 128], BF16, isOutput=False)
    maskm = nc.declare_dram_parameter("maskm", [128, 128], BF16, isOutput=False)
    onec = nc.declare_dram_parameter("onec", [128, 64], F32R, isOutput=False)
    one8 = nc.declare_dram_parameter("one8", [128, 8], F8, isOutput=False)
    mskc = nc.declare_dram_parameter("mskc", [64, 1], F32, isOutput=False)
    yout = nc.declare_dram_parameter("yout", [T, C], F32, isOutput=True)

    with ExitStack() as ctx:
        ctx.enter_context(nc.allow_low_precision(
            reason="fp8 PV with residual correction; bf16 scores; fp32r elsewhere"))
        tc = ctx.enter_context(tile.TileContext(nc, pool_alloc_mode="queue"))

        # ---- persistent pools ----
        consts = ctx.enter_context(tc.tile_pool(name="consts", bufs=1))
        idn_sb = consts.tile([128, 128], BF16)
        maskm_sb = consts.tile([128, 128], BF16)
        msk_sb = consts.tile([64, 1], F32)
        bias_sb = consts.tile([128, 1], F32)
        nc.sync.dma_start(idn_sb[:], idn[:])
        nc.sync.dma_start(maskm_sb[:], maskm[:])
        nc.sync.dma_start(msk_sb[:], mskc[:])
        nc.gpsimd.memset(bias_sb[:], BIAS)

        qk_pool = ctx.enter_context(tc.tile_pool(name="qk_pool", bufs=1))
        qkT = [qk_pool.tile([128, T], BF16, name=f"qkT{fi}") for fi in range(8)]
        v0_pool = ctx.enter_context(tc.tile_pool(name="v0_pool", bufs=1))
        vaug0 = [v0_pool.tile([128, NH * 65], F32R, name=f"vaug0{tt}")
                 for tt in range(4)]
        vdr_pool = ctx.enter_context(tc.tile_pool(name="vdr_pool", bufs=1))
        vdr = [vdr_pool.tile([128, 2, NH, 128], F8, name=f"vdr{g}")
               for g in range(KC // 2)]
        wp_pool = ctx.enter_context(tc.tile_pool(name="wp_pool", bufs=1))
        wp_sb = [wp_pool.tile([128, C], F32R, name=f"wp{pp}") for pp in range(4)]
        for pp in range(4):
            nc.sync.dma_start(wp_sb[pp][:], wp[pp, :, :])
        # ones slots are never overwritten by the per-rep v writes: init once
        for g in range(KC // 2):
            for ko in range(2):
                nc.sync.dma_start(vdr[g][:, ko, :, 127:128], one8[:, 0:8])
        for tt in range(4):
            va = vaug0[tt].rearrange("p (h s) -> p h s", s=65)
            nc.sync.dma_start(va[:, :, 64:65], onec[:, 0:8])

        w1 = ctx.enter_context(tc.tile_pool(name="w1", bufs=1))
        wqk_sb = [w1.tile([128, 2 * CL], BF16, name=f"wqk{ci}") for ci in range(8)]
        wv_sb = [w1.tile([128, CL], BF16, name=f"wv{ci}") for ci in range(8)]
        for ci in range(8):
            nc.sync.dma_start(wqk_sb[ci][:], wqk[ci * 128:(ci + 1) * 128, :])
            nc.sync.dma_start(wv_sb[ci][:], wv[ci * 128:(ci + 1) * 128, :])

        loop = tc.For_i(0, loop_reps) if loop_reps is not None else None
        if loop is not None:
            ctx.enter_context(loop)

        with tc.tile_pool(name="xp", bufs=16) as xp, \
             tc.tile_pool(name="ps1", bufs=2, space="PSUM") as ps1, \
             tc.tile_pool(name="ap0", bufs=2) as ap0, \
             tc.tile_pool(name="ap8", bufs=6) as ap8, \
             tc.tile_pool(name="ysbp", bufs=6) as ysbp, \
             tc.tile_pool(name="cmb", bufs=3) as cmb, \
             tc.tile_pool(name="osb", bufs=2) as osbp, \
             tc.tile_pool(name="sps", bufs=2, space="PSUM") as sps, \
             tc.tile_pool(name="yps", bufs=2, space="PSUM") as yps:
            def load_x(tj):
                xt = []
                for ci in range(8):
                    t_ = xp.tile([128, 512], BF16, name="xt", tag="xt")
                    nc.sync.dma_start(t_[:], xT[ci * 128:(ci + 1) * 128,
                                                 tj * 512:(tj + 1) * 512])
                    xt.append(t_)
                return xt

            def qkv_slice(tj, xt, sl):
                """Emit qkv work slice sl (0..3) for token chunk tj."""
                for fi in (2 * sl, 2 * sl + 1):
                    ps = ps1.tile([128, 512], F32, name="qkps", tag="qkps")
                    for ci in range(8):
                        nc.tensor.matmul(
                            ps[:],
                            (wqk_sb[ci][:, fi * 128:(fi + 1) * 128]),
                            (xt[ci][:]),
                            start=(ci == 0), stop=(ci == 7))
                    nc.vector.tensor_copy(qkT[fi][:, tj * 512:(tj + 1) * 512], ps[:])
                ts = sl
                tt = tj * 4 + ts
                ps = ps1.tile([128, 512], F32, name="vps", tag="qkps")
                for ci in range(8):
                    nc.tensor.matmul(
                        ps[:],
                        (xt[ci][:, ts * 128:(ts + 1) * 128]),
                        (wv_sb[ci][:]),
                        start=(ci == 0), stop=(ci == 7))
                ps3 = ps.rearrange("p (h s) -> p h s", s=64)
                g, ko = tt // 2, tt % 2
                vg = vdr[g]
                nc.vector.tensor_copy(vg[:, ko, :, 0:64], ps3[:])
                nc.vector.tensor_sub(vg[:, ko, :, 64:127],
                                     ps3[:, :, 0:63], vg[:, ko, :, 0:63])
                if tt < 4:
                    va = vaug0[tt].rearrange("p (h s) -> p h s", s=65)
                    nc.vector.tensor_copy(va[:, :, 0:64], ps3[:])

            # chunk 0 computed up front; chunk j+1 interleaved into block j
            xt_cur = load_x(0)
            for sl in range(4):
                qkv_slice(0, xt_cur, sl)
            for j in range(TJ):
                if j < TJ - 1:
                    xt_nxt = load_x(j + 1)
                yts = []
                for p in range(4):
                    pair = (2 * p, 2 * p + 1)
                    yt = ysbp.tile([128, 512], F32R, name="yt", tag="yt")
                    yts.append(yt)
                    att = {}
                    for kcg in range(2 * (j + 1)):
                        sp = {h: sps.tile([128, 1024], F32, name="sp", tag="sp")
                              for h in pair}
                        for u in range(2):
                            kc = 2 * kcg + u
                            d = max(0, (kc - 4 * j) * 128)
                            for h in pair:
                                base = (h % 2) * 64
                                ksl = qkT[4 + h // 2][base:base + 64,
                                                     kc * 128:(kc + 1) * 128]
                                qsl = qkT[h // 2][base:base + 64,
                                                  j * 512 + d:(j + 1) * 512]
                                nc.tensor.matmul(
                                    sp[h][:, u * 512 + d:(u + 1) * 512],
                                    (ksl), (qsl),
                                    start=True, stop=(kc < 4 * j),
                                    skip_group_check=True)
                            if kc >= 4 * j:
                                for h in pair:
                                    nc.tensor.matmul(
                                        sp[h][:, u * 512 + d:u * 512 + d + 128],
                                        idn_sb[:], maskm_sb[:],
                                        start=False, stop=True,
                                        skip_group_check=True)
                        lo = 256 if (j > 0 and kcg == 2 * j + 1) else 0
                        for h in pair:
                            if j == 0:
                                at = ap0.tile([128, 1024], F32R, name="at0",
                                              tag="at0")
                                nc.scalar.activation(at[:], sp[h][:], EXP,
                                                     scale=SCALE)
                            else:
                                at = ap8.tile([128, 1024], F8, name="at8",
                                              tag="at8")
                                nc.scalar.activation(
                                    at[:, lo:1024], sp[h][:, lo:1024], EXP,
                                    scale=SCALE, bias=bias_sb[:])
                            att[(h, kcg)] = at
                    # PV + combine, odd head first so its remap DMA overlaps
                    for h in (pair[1], pair[0]):
                        yp = yps.tile([128, 512], F32, name="yp", tag="yp")
                        if j == 0:
                            for kcg in range(2):
                                for u in range(2):
                                    kc = 2 * kcg + u
                                    d = kc * 128
                                    nc.tensor.matmul(
                                        yp[0:65, d:512],
                                        (vaug0[kc][:, h * 65:h * 65 + 65]),
                                        (att[(h, kcg)][:, u * 512 + d:(u + 1) * 512]),
                                        start=(kc == 0), stop=(kc == 3),
                                        skip_group_check=True)
                        else:
                            for kcg in range(2 * (j + 1)):
                                at3 = att[(h, kcg)].rearrange(
                                    "p (ko q) -> p ko q", q=512)
                                kc0, kc1 = 2 * kcg, 2 * kcg + 1
                                d0 = max(0, (kc0 - 4 * j) * 128)
                                d1 = max(0, (kc1 - 4 * j) * 128)
                                if d1 > d0:
                                    nc.tensor.matmul(
                                        yp[:, d0:d1],
                                        vdr[kcg][:, 0, h, :],
                                        at3[:, 0, d0:d1],
                                        start=False, stop=False,
                                        skip_group_check=True)
                                nc.tensor.matmul(
                                    yp[:, d1:512],
                                    vdr[kcg][:, :, h, :],
                                    at3[:, :, d1:512],
                                    start=(kcg == 0), stop=(kcg == 2 * j + 1),
                                    perf_mode=DR, skip_group_check=True)
                        # ---- combine + normalize ----
                        dn = cmb.tile([1, 512], F32, name="dn", tag="dn")
                        rc = cmb.tile([1, 512], F32, name="rc", tag="rc")
                        bp = cmb.tile([64, 512], F32, name="bp", tag="bp")
                        if j == 0:
                            sc = cmb.tile([128, 512], F32, name="sc0", tag="sc0")
                            nc.vector.tensor_copy(sc[64:65, :], yp[64:65, :])
                            nc.scalar.dma_start(dn[:], sc[64:65, :])
                            nc.vector.reciprocal(rc[:], dn[:])
                            nc.gpsimd.partition_broadcast(bp[:], rc[:])
                            if h % 2 == 0:
                                nc.vector.tensor_mul(yt[0:64, :], yp[0:64, :], bp[:])
                            else:
                                ytmp = cmb.tile([64, 512], F32R, name="ytmp",
                                                tag="ytmp")
                                nc.vector.tensor_mul(ytmp[:], yp[0:64, :], bp[:])
                                nc.scalar.dma_start(yt[64:128, :], ytmp[:])
                        else:
                            sc = cmb.tile([128, 512], F32, name="sc", tag="sc0")
                            nc.vector.tensor_copy(sc[64:128, :], yp[64:128, :])
                            corr = cmb.tile([64, 512], F32, name="corr", tag="corr")
                            nc.scalar.dma_start(corr[:], sc[64:128, :])
                            nc.scalar.dma_start(dn[:], sc[127:128, :])
                            nc.vector.reciprocal(rc[:], dn[:])
                            nc.gpsimd.partition_broadcast(bp[:], rc[:])
                            tsum = cmb.tile([64, 512], F32, name="tsum", tag="tsum")
                            nc.vector.scalar_tensor_tensor(
                                tsum[:], corr[:], msk_sb[:], yp[0:64, :],
                                op0=MUL, op1=ADD)
                            if h % 2 == 0:
                                nc.vector.tensor_mul(yt[0:64, :], tsum[:], bp[:])
                            else:
                                ytmp = cmb.tile([64, 512], F32R, name="ytmp",
                                                tag="ytmp")
                                nc.vector.tensor_mul(ytmp[:], tsum[:], bp[:])
                                nc.scalar.dma_start(yt[64:128, :], ytmp[:])
                    # interleaved qkv slice for the next token chunk
                    if j < TJ - 1:
                        qkv_slice(j + 1, xt_nxt, p)
                # ---- proj for this token block ----
                for ts in range(4):
                    ot = osbp.tile([128, 1024], F32, name="ot", tag="ot")
                    for co in range(2):
                        ps = yps.tile([128, 512], F32, name="pps", tag="yp")
                        for pp in range(4):
                            nc.tensor.matmul(
                                ps[:],
                                (yts[pp][:, ts * 128:(ts + 1) * 128]),
                                (wp_sb[pp][:, co * 512:(co + 1) * 512]),
                                start=(pp == 0), stop=(pp == 3))
                        nc.vector.tensor_copy(ot[:, co * 512:(co + 1) * 512], ps[:])
                    nc.sync.dma_start(
                        yout[(j * 4 + ts) * 128:(j * 4 + ts + 1) * 128, :], ot[:])

    nc.compile()
    return nc


# ---------------- host-side sharding ----------------

def shard_inputs(x, w_qkv, w_proj):
    """Full inputs -> list of 8 per-core input maps."""
    import ml_dtypes
    idn = np.eye(128, dtype=ml_dtypes.bfloat16)
    r = np.arange(128)
    maskm = np.where(r[:, None] > r[None, :], -1e9, 0.0).astype(ml_dtypes.bfloat16)
    one8 = np.ones((128, 8), ml_dtypes.float8_e4m3)
    msk = np.ones((64, 1), np.float32)
    msk[63] = 0.0
    in_maps = []
    for core in range(8):
        b, g = core // 2, core % 2
        sl = slice(g * CL, (g + 1) * CL)
        in_maps.append(dict(
            xT=np.ascontiguousarray(x[b].T).astype(ml_dtypes.bfloat16),
            wqk=np.ascontiguousarray(
                np.concatenate([w_qkv[:, sl], w_qkv[:, C + g * CL:C + (g + 1) * CL]],
                               axis=1)).astype(ml_dtypes.bfloat16),
            wv=np.ascontiguousarray(w_qkv[:, 2 * C + g * CL:2 * C + (g + 1) * CL]).astype(ml_dtypes.bfloat16),
            wp=np.ascontiguousarray(w_proj[sl, :].reshape(4, 128, C)),
            idn=idn, maskm=maskm, onec=np.ones((128, 64), np.float32),
            one8=one8, mskc=msk,
        ))
    return in_maps


def unshard_output(results, b_proj):
    """Per-core partial [T, C] projections -> full [B, T, C] output."""
    out = np.empty((4, T, C), dtype=np.float32)
    for b in range(4):
        out[b] = results[2 * b]["yout"] + results[2 * b + 1]["yout"]
    out += b_proj[None, None, :]
    return out


_CACHE = {}


def kernel(x, w_qkv, w_proj, b_proj):
    from concourse.bass_utils import run_bass_kernel_spmd
    if "nc" not in _CACHE:
        _CACHE["nc"] = build_nc()
    nc = _CACHE["nc"]
    in_maps = shard_inputs(np.asarray(x, np.float32),
                           np.asarray(w_qkv, np.float32),
                           np.asarray(w_proj, np.float32))
    res = run_bass_kernel_spmd(nc, in_maps, core_ids=list(range(8)))
    return unshard_output(res.results, np.asarray(b_proj, np.float32))



# revision 17
# speedup vs baseline: 1.0861x; 1.0861x over previous
"""v5: ACT-queue unblocking. Causal self-attention (B=4, T=2048, C=1024,
H=16) on 8 TRN2 NeuronCores.

Sharding (tensor-parallel over batch x head-group): core c handles batch c//2
and heads [8*(c%2), 8*(c%2)+8). Host sums the two head-group partials per
batch and adds the bias.

v5 on top of v4: the sim trace showed ScalarE (exp) is the bottleneck engine
and the combine DMAs dispatched from its queue stall it ~2.4us per head
while they wait for their source. Changes:
  - dn DMAs eliminated: reciprocal runs directly on the denominator
    partition (p64 for j=0, p127/p63 for j>=1) and partition_broadcast
    sources that partition; nothing is remapped to partition 0.
  - odd heads' vdr layout is [resid 63 | 1 | v8 64] so their PV output
    lands on partitions 64:128 directly -> no ytmp remap DMA for j>=1.
  - remaining remap DMAs (corr, j=0 odd ytmp) moved to the gpsimd
    (SWDGE) queue; the ACT queue carries no per-iteration DMAs at all.
  - startup weight/const DMAs moved to the ACT queue so the SP queue
    starts the x chunk-0 loads immediately.
"""
from contextlib import ExitStack

import numpy as np
import concourse.bass as bass
import concourse.mybir as mybir
import concourse.tile as tile
from concourse import bacc

F32 = mybir.dt.float32
F32R = mybir.dt.float32r
BF16 = mybir.dt.bfloat16
F8 = mybir.dt.float8e4
EXP = mybir.ActivationFunctionType.Exp
DR = mybir.MatmulPerfMode.DoubleRow
MUL = mybir.AluOpType.mult
ADD = mybir.AluOpType.add

T = 2048          # tokens
C = 1024          # channels
NH = 8            # local heads
HD = 64           # head dim
CL = NH * HD      # local channels (512)
TJ = T // 512     # 4 q-chunks of 512
KC = T // 128     # 16 k-chunks of 128
SCALE = HD ** -0.5
BIAS = -3.0       # exp bias for fp8 att storage (cancels in normalization)


def build_nc(loop_reps: int | None = None):
    nc = bacc.Bacc("TRN2", target_bir_lowering=False, debug=False)
    xT = nc.declare_dram_parameter("xT", [C, T], BF16, isOutput=False)
    wqk = nc.declare_dram_parameter("wqk", [C, 2 * CL], BF16, isOutput=False)
    wv = nc.declare_dram_parameter("wv", [C, CL], BF16, isOutput=False)
    wp = nc.declare_dram_parameter("wp", [4, 128, C], F32R, isOutput=False)
    idn = nc.declare_dram_parameter("idn", [128, 128], BF16, isOutput=False)
    maskm = nc.declare_dram_parameter("maskm", [128, 128], BF16, isOutput=False)
    onec = nc.declare_dram_parameter("onec", [128, 64], F32R, isOutput=False)
    one8 = nc.declare_dram_parameter("one8", [128, 8], F8, isOutput=False)
    mskc = nc.declare_dram_parameter("mskc", [128, 1], F32, isOutput=False)
    yout = nc.declare_dram_parameter("yout", [T, C], F32, isOutput=True)

    with ExitStack() as ctx:
        ctx.enter_context(nc.allow_low_precision(
            reason="fp8 PV with residual correction; bf16 scores; fp32r elsewhere"))
        tc = ctx.enter_context(tile.TileContext(nc, pool_alloc_mode="queue"))

        # ---- persistent pools ----
        # weight loads go on the ACT queue (idle until the first exp) so the
        # SP queue reaches the per-rep x loads immediately.
        w1 = ctx.enter_context(tc.tile_pool(name="w1", bufs=1))
        wqk_sb = [w1.tile([128, 2 * CL], BF16, name=f"wqk{ci}") for ci in range(8)]
        wv_sb = [w1.tile([128, CL], BF16, name=f"wv{ci}") for ci in range(8)]
        for ci in range(8):
            nc.scalar.dma_start(wqk_sb[ci][:], wqk[ci * 128:(ci + 1) * 128, :])
        for ci in range(8):
            nc.scalar.dma_start(wv_sb[ci][:], wv[ci * 128:(ci + 1) * 128, :])

        consts = ctx.enter_context(tc.tile_pool(name="consts", bufs=1))
        idn_sb = consts.tile([128, 128], BF16)
        maskm_sb = consts.tile([128, 128], BF16)
        msk_sb = consts.tile([128, 1], F32)
        bias_sb = consts.tile([128, 1], F32)
        nc.scalar.dma_start(idn_sb[:], idn[:])
        nc.scalar.dma_start(maskm_sb[:], maskm[:])
        nc.scalar.dma_start(msk_sb[:], mskc[:])
        nc.gpsimd.memset(bias_sb[:], BIAS)

        qk_pool = ctx.enter_context(tc.tile_pool(name="qk_pool", bufs=1))
        qkT = [qk_pool.tile([128, T], BF16, name=f"qkT{fi}") for fi in range(8)]
        v0_pool = ctx.enter_context(tc.tile_pool(name="v0_pool", bufs=1))
        vaug0 = [v0_pool.tile([128, NH * 65], F32R, name=f"vaug0{tt}")
                 for tt in range(4)]
        vdr_pool = ctx.enter_context(tc.tile_pool(name="vdr_pool", bufs=1))
        vdr = [vdr_pool.tile([128, 2, NH, 128], F8, name=f"vdr{g}")
               for g in range(KC // 2)]
        wp_pool = ctx.enter_context(tc.tile_pool(name="wp_pool", bufs=1))
        wp_sb = [wp_pool.tile([128, C], F32R, name=f"wp{pp}") for pp in range(4)]
        for pp in range(4):
            nc.scalar.dma_start(wp_sb[pp][:], wp[pp, :, :])
        # persistent corr staging tile; rows 63/127 are never written by the
        # remap DMAs but are read (x msk==0), so zero them once.
        corr_pool = ctx.enter_context(tc.tile_pool(name="corr_pool", bufs=1))
        corr_sb = corr_pool.tile([128, 512], F32, name="corr_sb")
        nc.gpsimd.memset(corr_sb[32:64, :], 0.0)
        nc.gpsimd.memset(corr_sb[96:128, :], 0.0)
        # ones slots are never overwritten by the per-rep v writes: init once.
        # even heads use [v8 64 | resid 63 | 1@127] (y8@0:64, dn@p127);
        # odd heads use [resid 63 | 1@63 | v8 64] (dn@p63, y8@p64:128) so PV
        # puts their y on partitions 64:128 directly. All fp8 DVE writes
        # stay at byte offsets 0/64 within the head segment.
        # DVE writes (same queue as the per-rep v writes, byte-granular)
        # instead of 1-byte DMAs: the DMA path does sub-word
        # read-modify-write on SBUF which races with the first rep's DVE
        # writes to neighboring bytes of the same word.
        ones32 = consts.tile([128, 8], F32)
        nc.gpsimd.memset(ones32[:], 1.0)
        for g in range(KC // 2):
            vg4 = vdr[g].rearrange("p ko (pr par) c -> p ko pr par c", par=2)
            for ko in range(2):
                nc.vector.tensor_copy(vg4[:, ko, :, 0, 127:128], ones32[:, 0:4])
                nc.vector.tensor_copy(vg4[:, ko, :, 1, 63:64], ones32[:, 4:8])
        for tt in range(4):
            va = vaug0[tt].rearrange("p (h s) -> p h s", s=65)
            nc.vector.tensor_copy(va[:, :, 64:65], ones32[:, 0:8])

        loop = tc.For_i(0, loop_reps) if loop_reps is not None else None
        if loop is not None:
            ctx.enter_context(loop)

        with tc.tile_pool(name="xp", bufs=16) as xp, \
             tc.tile_pool(name="ps1", bufs=2, space="PSUM") as ps1, \
             tc.tile_pool(name="ap0", bufs=2) as ap0, \
             tc.tile_pool(name="ap8", bufs=6) as ap8, \
             tc.tile_pool(name="ysbp", bufs=6) as ysbp, \
             tc.tile_pool(name="cmb", bufs=3) as cmb, \
             tc.tile_pool(name="osb", bufs=2) as osbp, \
             tc.tile_pool(name="sps", bufs=2, space="PSUM") as sps, \
             tc.tile_pool(name="yps", bufs=2, space="PSUM") as yps:
            def load_x(tj):
                xt = []
                for ci in range(8):
                    t_ = xp.tile([128, 512], BF16, name="xt", tag="xt")
                    nc.sync.dma_start(t_[:], xT[ci * 128:(ci + 1) * 128,
                                                 tj * 512:(tj + 1) * 512])
                    xt.append(t_)
                return xt

            def qkv_slice(tj, xt, sl):
                """Emit qkv work slice sl (0..3) for token chunk tj."""
                for fi in (2 * sl, 2 * sl + 1):
                    ps = ps1.tile([128, 512], F32, name="qkps", tag="qkps")
                    for ci in range(8):
                        nc.tensor.matmul(
                            ps[:],
                            (wqk_sb[ci][:, fi * 128:(fi + 1) * 128]),
                            (xt[ci][:]),
                            start=(ci == 0), stop=(ci == 7))
                    nc.vector.tensor_copy(qkT[fi][:, tj * 512:(tj + 1) * 512], ps[:])
                ts = sl
                tt = tj * 4 + ts
                ps = ps1.tile([128, 512], F32, name="vps", tag="qkps")
                for ci in range(8):
                    nc.tensor.matmul(
                        ps[:],
                        (xt[ci][:, ts * 128:(ts + 1) * 128]),
                        (wv_sb[ci][:]),
                        start=(ci == 0), stop=(ci == 7))
                ps4 = ps.rearrange("p (pr par s) -> p pr par s", par=2, s=64)
                g, ko = tt // 2, tt % 2
                vg4 = vdr[g].rearrange("p ko (pr par) c -> p ko pr par c", par=2)
                # even heads: [v8 | resid | 1@127]
                nc.vector.tensor_copy(vg4[:, ko, :, 0, 0:64], ps4[:, :, 0, :])
                nc.vector.tensor_sub(vg4[:, ko, :, 0, 64:127],
                                     ps4[:, :, 0, 0:63], vg4[:, ko, :, 0, 0:63])
                # odd heads: [resid | 1@63 | v8]
                nc.vector.tensor_copy(vg4[:, ko, :, 1, 64:128], ps4[:, :, 1, :])
                nc.vector.tensor_sub(vg4[:, ko, :, 1, 0:63],
                                     ps4[:, :, 1, 0:63], vg4[:, ko, :, 1, 64:127])
                if tt < 4:
                    va = vaug0[tt].rearrange("p (h s) -> p h s", s=65)
                    nc.vector.tensor_copy(va[:, :, 0:64],
                                          ps.rearrange("p (h s) -> p h s", s=64)[:])

            # chunk 0 computed up front; chunk j+1 interleaved into block j
            xt_cur = load_x(0)
            for sl in range(4):
                qkv_slice(0, xt_cur, sl)
            for j in range(TJ):
                if j < TJ - 1:
                    xt_nxt = load_x(j + 1)
                yts = []
                for p in range(4):
                    pair = (2 * p, 2 * p + 1)
                    yt = ysbp.tile([128, 512], F32R, name="yt", tag="yt")
                    yts.append(yt)
                    att = {}
                    for kcg in range(2 * (j + 1)):
                        sp = {h: sps.tile([128, 1024], F32, name="sp", tag="sp")
                              for h in pair}
                        for u in range(2):
                            kc = 2 * kcg + u
                            d = max(0, (kc - 4 * j) * 128)
                            for h in pair:
                                base = (h % 2) * 64
                                ksl = qkT[4 + h // 2][base:base + 64,
                                                     kc * 128:(kc + 1) * 128]
                                qsl = qkT[h // 2][base:base + 64,
                                                  j * 512 + d:(j + 1) * 512]
                                nc.tensor.matmul(
                                    sp[h][:, u * 512 + d:(u + 1) * 512],
                                    (ksl), (qsl),
                                    start=True, stop=(kc < 4 * j),
                                    skip_group_check=True)
                            if kc >= 4 * j:
                                for h in pair:
                                    nc.tensor.matmul(
                                        sp[h][:, u * 512 + d:u * 512 + d + 128],
                                        idn_sb[:], maskm_sb[:],
                                        start=False, stop=True,
                                        skip_group_check=True)
                        # exp regions skip the never-written diagonal holes
                        # (keeps the PSUM reads fully initialized)
                        if kcg == 2 * j:
                            regs = ((0, 512), (640, 1024))
                        elif kcg == 2 * j + 1:
                            regs = ((256, 512), (896, 1024))
                        else:
                            regs = ((0, 1024),)
                        for h in pair:
                            if j == 0:
                                at = ap0.tile([128, 1024], F32R, name="at0",
                                              tag="at0")
                                for (a, b) in regs:
                                    nc.scalar.activation(
                                        at[:, a:b], sp[h][:, a:b], EXP,
                                        scale=SCALE)
                            else:
                                at = ap8.tile([128, 1024], F8, name="at8",
                                              tag="at8")
                                for (a, b) in regs:
                                    nc.scalar.activation(
                                        at[:, a:b], sp[h][:, a:b], EXP,
                                        scale=SCALE, bias=bias_sb[:])
                            att[(h, kcg)] = at
                    # PV + combine. odd head's y lands on partitions 64:128
                    # (vdr layout) so no remap DMA is needed for j>=1.
                    for h in (pair[1], pair[0]):
                        yp = yps.tile([128, 512], F32, name="yp", tag="yp")
                        if j == 0:
                            for kcg in range(2):
                                for u in range(2):
                                    kc = 2 * kcg + u
                                    d = kc * 128
                                    nc.tensor.matmul(
                                        yp[0:65, d:512],
                                        (vaug0[kc][:, h * 65:h * 65 + 65]),
                                        (att[(h, kcg)][:, u * 512 + d:(u + 1) * 512]),
                                        start=(kc == 0), stop=(kc == 3),
                                        skip_group_check=True)
                        else:
                            for kcg in range(2 * (j + 1)):
                                at3 = att[(h, kcg)].rearrange(
                                    "p (ko q) -> p ko q", q=512)
                                kc0, kc1 = 2 * kcg, 2 * kcg + 1
                                d0 = max(0, (kc0 - 4 * j) * 128)
                                d1 = max(0, (kc1 - 4 * j) * 128)
                                if d1 > d0:
                                    nc.tensor.matmul(
                                        yp[:, d0:d1],
                                        vdr[kcg][:, 0, h, :],
                                        at3[:, 0, d0:d1],
                                        start=False, stop=False,
                                        skip_group_check=True)
                                nc.tensor.matmul(
                                    yp[:, d1:512],
                                    vdr[kcg][:, :, h, :],
                                    at3[:, :, d1:512],
                                    start=(kcg == 0), stop=(kcg == 2 * j + 1),
                                    perf_mode=DR, skip_group_check=True)
                        # ---- combine + normalize (no ACT-queue DMAs).
                        # partition_broadcast on HW needs src AND dst at
                        # partition 0, so dn is routed through p0; odd heads
                        # get that free from their vdr layout.
                        rc = cmb.tile([128, 512], F32, name="rc", tag="rc")
                        bp = cmb.tile([128, 512], F32, name="bp", tag="bp")
                        if j == 0:
                            # y8 @ 0:64, dn @ 64 for both parities
                            dnc = cmb.tile([128, 512], F32, name="dnc",
                                           tag="sc0")
                            dn0 = cmb.tile([1, 512], F32, name="dn0",
                                           tag="dn0")
                            nc.vector.tensor_copy(dnc[64:65, :], yp[64:65, :])
                            nc.sync.dma_start(dn0[0:1, :], dnc[64:65, :])
                            nc.vector.reciprocal(rc[0:1, :], dn0[0:1, :])
                            nc.gpsimd.partition_broadcast(bp[0:64, :],
                                                          rc[0:1, :])
                            if h % 2 == 0:
                                nc.vector.tensor_mul(yt[0:64, :], yp[0:64, :],
                                                     bp[0:64, :])
                            else:
                                ytmp = cmb.tile([64, 512], F32R, name="ytmp",
                                                tag="ytmp")
                                nc.vector.tensor_mul(ytmp[:], yp[0:64, :],
                                                     bp[0:64, :])
                                nc.sync.dma_start(yt[64:128, :], ytmp[:])
                        else:
                            sc = cmb.tile([128, 512], F32, name="sc", tag="sc0")
                            tsum = cmb.tile([128, 512], F32, name="tsum",
                                            tag="tsum")
                            # full-width copy frees yp after one DVE op; the
                            # corr remap + tsum then read sc, not PSUM.
                            nc.vector.tensor_copy(sc[:, :], yp[:, :])
                            dn0 = cmb.tile([1, 512], F32, name="dn0",
                                           tag="dn0")
                            if h % 2 == 0:
                                # y8@0:64, resid@64:127, dn@127
                                nc.sync.dma_start(corr_sb[0:63, :],
                                                  sc[64:127, :])
                                nc.sync.dma_start(dn0[0:1, :], sc[127:128, :])
                                nc.vector.reciprocal(rc[0:1, :], dn0[0:1, :])
                                nc.gpsimd.partition_broadcast(bp[0:64, :],
                                                              rc[0:1, :])
                                # corr_sb[63] is memset 0, so a plain add
                                # applies the residual correction
                                nc.vector.tensor_add(
                                    tsum[0:64, :], corr_sb[0:64, :],
                                    sc[0:64, :])
                                nc.vector.tensor_mul(yt[0:64, :], tsum[0:64, :],
                                                     bp[0:64, :])
                            else:
                                # resid@0:63, dn@63, y8@64:128
                                nc.sync.dma_start(corr_sb[64:127, :],
                                                  sc[0:63, :])
                                nc.sync.dma_start(dn0[0:1, :], sc[63:64, :])
                                nc.vector.reciprocal(rc[0:1, :], dn0[0:1, :])
                                nc.gpsimd.partition_broadcast(bp[0:128, :],
                                                              rc[0:1, :])
                                nc.vector.tensor_add(
                                    tsum[64:128, :], corr_sb[64:128, :],
                                    sc[64:128, :])
                                nc.vector.tensor_mul(yt[64:128, :],
                                                     tsum[64:128, :],
                                                     bp[64:128, :])
                    # interleaved qkv slice for the next token chunk
                    if j < TJ - 1:
                        qkv_slice(j + 1, xt_nxt, p)
                # ---- proj for this token block ----
                for ts in range(4):
                    ot = osbp.tile([128, 1024], F32, name="ot", tag="ot")
                    for co in range(2):
                        ps = yps.tile([128, 512], F32, name="pps", tag="yp")
                        for pp in range(4):
                            nc.tensor.matmul(
                                ps[:],
                                (yts[pp][:, ts * 128:(ts + 1) * 128]),
                                (wp_sb[pp][:, co * 512:(co + 1) * 512]),
                                start=(pp == 0), stop=(pp == 3))
                        nc.vector.tensor_copy(ot[:, co * 512:(co + 1) * 512], ps[:])
                    nc.sync.dma_start(
                        yout[(j * 4 + ts) * 128:(j * 4 + ts + 1) * 128, :], ot[:])

    nc.compile()
    return nc


# ---------------- host-side sharding ----------------

def shard_inputs(x, w_qkv, w_proj):
    """Full inputs -> list of 8 per-core input maps."""
    import ml_dtypes
    idn = np.eye(128, dtype=ml_dtypes.bfloat16)
    r = np.arange(128)
    maskm = np.where(r[:, None] > r[None, :], -1e9, 0.0).astype(ml_dtypes.bfloat16)
    one8 = np.ones((128, 8), ml_dtypes.float8_e4m3)
    msk = np.ones((128, 1), np.float32)
    msk[63] = 0.0
    msk[127] = 0.0
    in_maps = []
    for core in range(8):
        b, g = core // 2, core % 2
        sl = slice(g * CL, (g + 1) * CL)
        in_maps.append(dict(
            xT=np.ascontiguousarray(x[b].T).astype(ml_dtypes.bfloat16),
            wqk=np.ascontiguousarray(
                np.concatenate([w_qkv[:, sl], w_qkv[:, C + g * CL:C + (g + 1) * CL]],
                               axis=1)).astype(ml_dtypes.bfloat16),
            wv=np.ascontiguousarray(w_qkv[:, 2 * C + g * CL:2 * C + (g + 1) * CL]).astype(ml_dtypes.bfloat16),
            wp=np.ascontiguousarray(w_proj[sl, :].reshape(4, 128, C)),
            idn=idn, maskm=maskm, onec=np.ones((128, 64), np.float32),
            one8=one8, mskc=msk,
        ))
    return in_maps


def unshard_output(results, b_proj):
    """Per-core partial [T, C] projections -> full [B, T, C] output."""
    out = np.empty((4, T, C), dtype=np.float32)
    for b in range(4):
        out[b] = results[2 * b]["yout"] + results[2 * b + 1]["yout"]
    out += b_proj[None, None, :]
    return out


_CACHE = {}


def kernel(x, w_qkv, w_proj, b_proj):
    from concourse.bass_utils import run_bass_kernel_spmd
    if "nc" not in _CACHE:
        _CACHE["nc"] = build_nc()
    nc = _CACHE["nc"]
    in_maps = shard_inputs(np.asarray(x, np.float32),
                           np.asarray(w_qkv, np.float32),
                           np.asarray(w_proj, np.float32))
    res = run_bass_kernel_spmd(nc, in_maps, core_ids=list(range(8)))
    return unshard_output(res.results, np.asarray(b_proj, np.float32))


# revision 19
# speedup vs baseline: 1.0997x; 1.0125x over previous
"""v5.6: v5.4 + DVE trimask (mask matmuls removed). Causal self-attention (B=4, T=2048, C=1024,
H=16) on 8 TRN2 NeuronCores.

Sharding (tensor-parallel over batch x head-group): core c handles batch c//2
and heads [8*(c%2), 8*(c%2)+8). Host sums the two head-group partials per
batch and adds the bias.

v5 on top of v4: the sim trace showed ScalarE (exp) is the bottleneck engine
and the combine DMAs dispatched from its queue stall it ~2.4us per head
while they wait for their source. Changes:
  - dn DMAs eliminated: reciprocal runs directly on the denominator
    partition (p64 for j=0, p127/p63 for j>=1) and partition_broadcast
    sources that partition; nothing is remapped to partition 0.
  - odd heads' vdr layout is [resid 63 | 1 | v8 64] so their PV output
    lands on partitions 64:128 directly -> no ytmp remap DMA for j>=1.
  - remaining remap DMAs (corr, j=0 odd ytmp) moved to the gpsimd
    (SWDGE) queue; the ACT queue carries no per-iteration DMAs at all.
  - startup weight/const DMAs moved to the ACT queue so the SP queue
    starts the x chunk-0 loads immediately.
"""
from contextlib import ExitStack

import numpy as np
import concourse.bass as bass
import concourse.mybir as mybir
import concourse.tile as tile
from concourse import bacc

F32 = mybir.dt.float32
F32R = mybir.dt.float32r
BF16 = mybir.dt.bfloat16
F8 = mybir.dt.float8e4
EXP = mybir.ActivationFunctionType.Exp
DR = mybir.MatmulPerfMode.DoubleRow
MUL = mybir.AluOpType.mult
ADD = mybir.AluOpType.add

T = 2048          # tokens
C = 1024          # channels
NH = 8            # local heads
HD = 64           # head dim
CL = NH * HD      # local channels (512)
TJ = T // 512     # 4 q-chunks of 512
KC = T // 128     # 16 k-chunks of 128
SCALE = HD ** -0.5
BIAS = -3.0       # exp bias for fp8 att storage (cancels in normalization)


def build_nc(loop_reps: int | None = None):
    nc = bacc.Bacc("TRN2", target_bir_lowering=False, debug=False)
    xT = nc.declare_dram_parameter("xT", [C, T], BF16, isOutput=False)
    wqk = nc.declare_dram_parameter("wqk", [C, 2 * CL], BF16, isOutput=False)
    wv = nc.declare_dram_parameter("wv", [C, CL], BF16, isOutput=False)
    wp = nc.declare_dram_parameter("wp", [4, 128, C], F32R, isOutput=False)
    trim = nc.declare_dram_parameter("trim", [128, 128], F32, isOutput=False)
    yout = nc.declare_dram_parameter("yout", [T, C], F32, isOutput=True)

    with ExitStack() as ctx:
        ctx.enter_context(nc.allow_low_precision(
            reason="fp8 PV with residual correction; bf16 scores; fp32r elsewhere"))
        tc = ctx.enter_context(tile.TileContext(nc, pool_alloc_mode="queue"))

        # ---- persistent pools ----
        # weight loads go on the ACT queue (idle until the first exp) so the
        # SP queue reaches the per-rep x loads immediately.
        w1 = ctx.enter_context(tc.tile_pool(name="w1", bufs=1))
        wqk_sb = [w1.tile([128, 2 * CL], BF16, name=f"wqk{ci}") for ci in range(8)]
        wv_sb = [w1.tile([128, CL], BF16, name=f"wv{ci}") for ci in range(8)]
        for ci in range(8):
            nc.scalar.dma_start(wqk_sb[ci][:], wqk[ci * 128:(ci + 1) * 128, :])
        for ci in range(8):
            nc.scalar.dma_start(wv_sb[ci][:], wv[ci * 128:(ci + 1) * 128, :])

        consts = ctx.enter_context(tc.tile_pool(name="consts", bufs=1))
        trim_f = consts.tile([128, 128], F32)
        trim8 = consts.tile([128, 128], F8)
        trimr = consts.tile([128, 128], F32R)
        bias_sb = consts.tile([128, 1], F32)
        nc.scalar.dma_start(trim_f[:], trim[:])
        nc.vector.tensor_copy(trim8[:], trim_f[:])
        nc.vector.tensor_copy(trimr[:], trim_f[:])
        nc.gpsimd.memset(bias_sb[:], BIAS)

        qk_pool = ctx.enter_context(tc.tile_pool(name="qk_pool", bufs=1))
        qkT = [qk_pool.tile([128, T], BF16, name=f"qkT{fi}") for fi in range(8)]
        v0_pool = ctx.enter_context(tc.tile_pool(name="v0_pool", bufs=1))
        vaug0 = [v0_pool.tile([128, NH * 65], F32R, name=f"vaug0{tt}")
                 for tt in range(4)]
        vdr_pool = ctx.enter_context(tc.tile_pool(name="vdr_pool", bufs=1))
        vdr = [vdr_pool.tile([128, 2, NH, 128], F8, name=f"vdr{g}")
               for g in range(KC // 2)]
        wp_pool = ctx.enter_context(tc.tile_pool(name="wp_pool", bufs=1))
        wp_sb = [wp_pool.tile([128, C], F32R, name=f"wp{pp}") for pp in range(4)]
        for pp in range(4):
            nc.scalar.dma_start(wp_sb[pp][:], wp[pp, :, :])
        # persistent corr staging tile; rows 63/127 are never written by the
        # remap DMAs but are read (x msk==0), so zero them once.
        corr_pool = ctx.enter_context(tc.tile_pool(name="corr_pool", bufs=1))
        corr_sb = corr_pool.tile([128, 512], F32, name="corr_sb")
        nc.gpsimd.memset(corr_sb[32:64, :], 0.0)
        nc.gpsimd.memset(corr_sb[96:128, :], 0.0)
        # ones slots are never overwritten by the per-rep v writes: init once.
        # even heads use [v8 64 | resid 63 | 1@127] (y8@0:64, dn@p127);
        # odd heads use [resid 63 | 1@63 | v8 64] (dn@p63, y8@p64:128) so PV
        # puts their y on partitions 64:128 directly. All fp8 DVE writes
        # stay at byte offsets 0/64 within the head segment.
        # DVE writes (same queue as the per-rep v writes, byte-granular)
        # instead of 1-byte DMAs: the DMA path does sub-word
        # read-modify-write on SBUF which races with the first rep's DVE
        # writes to neighboring bytes of the same word.
        ones32 = consts.tile([128, 8], F32)
        nc.gpsimd.memset(ones32[:], 1.0)
        for g in range(KC // 2):
            vg4 = vdr[g].rearrange("p ko (pr par) c -> p ko pr par c", par=2)
            for ko in range(2):
                nc.vector.tensor_copy(vg4[:, ko, :, 0, 127:128], ones32[:, 0:4])
                nc.vector.tensor_copy(vg4[:, ko, :, 1, 63:64], ones32[:, 4:8])
        for tt in range(4):
            va = vaug0[tt].rearrange("p (h s) -> p h s", s=65)
            nc.vector.tensor_copy(va[:, :, 64:65], ones32[:, 0:8])

        loop = tc.For_i(0, loop_reps) if loop_reps is not None else None
        if loop is not None:
            ctx.enter_context(loop)

        with tc.tile_pool(name="xp", bufs=16) as xp, \
             tc.tile_pool(name="ps1", bufs=2, space="PSUM") as ps1, \
             tc.tile_pool(name="ap0", bufs=2) as ap0, \
             tc.tile_pool(name="ap8", bufs=6) as ap8, \
             tc.tile_pool(name="ysbp", bufs=6) as ysbp, \
             tc.tile_pool(name="cmb", bufs=3) as cmb, \
             tc.tile_pool(name="osb", bufs=2) as osbp, \
             tc.tile_pool(name="sps", bufs=2, space="PSUM") as sps, \
             tc.tile_pool(name="yps", bufs=2, space="PSUM") as yps:
            def load_x(tj):
                xt = []
                for ci in range(8):
                    t_ = xp.tile([128, 512], BF16, name="xt", tag="xt")
                    nc.sync.dma_start(t_[:], xT[ci * 128:(ci + 1) * 128,
                                                 tj * 512:(tj + 1) * 512])
                    xt.append(t_)
                return xt

            def qkv_slice(tj, xt, sl):
                """Emit qkv work slice sl (0..3) for token chunk tj."""
                for fi in (2 * sl, 2 * sl + 1):
                    ps = ps1.tile([128, 512], F32, name="qkps", tag="qkps")
                    for ci in range(8):
                        nc.tensor.matmul(
                            ps[:],
                            (wqk_sb[ci][:, fi * 128:(fi + 1) * 128]),
                            (xt[ci][:]),
                            start=(ci == 0), stop=(ci == 7))
                    nc.vector.tensor_copy(qkT[fi][:, tj * 512:(tj + 1) * 512], ps[:])
                ts = sl
                tt = tj * 4 + ts
                ps = ps1.tile([128, 512], F32, name="vps", tag="qkps")
                for ci in range(8):
                    nc.tensor.matmul(
                        ps[:],
                        (xt[ci][:, ts * 128:(ts + 1) * 128]),
                        (wv_sb[ci][:]),
                        start=(ci == 0), stop=(ci == 7))
                ps4 = ps.rearrange("p (pr par s) -> p pr par s", par=2, s=64)
                g, ko = tt // 2, tt % 2
                vg4 = vdr[g].rearrange("p ko (pr par) c -> p ko pr par c", par=2)
                # even heads: [v8 | resid | 1@127]
                nc.vector.tensor_copy(vg4[:, ko, :, 0, 0:64], ps4[:, :, 0, :])
                nc.vector.tensor_sub(vg4[:, ko, :, 0, 64:127],
                                     ps4[:, :, 0, 0:63], vg4[:, ko, :, 0, 0:63])
                # odd heads: [resid | 1@63 | v8]
                nc.vector.tensor_copy(vg4[:, ko, :, 1, 64:128], ps4[:, :, 1, :])
                nc.vector.tensor_sub(vg4[:, ko, :, 1, 0:63],
                                     ps4[:, :, 1, 0:63], vg4[:, ko, :, 1, 64:127])
                if tt < 4:
                    va = vaug0[tt].rearrange("p (h s) -> p h s", s=65)
                    nc.vector.tensor_copy(va[:, :, 0:64],
                                          ps.rearrange("p (h s) -> p h s", s=64)[:])

            # chunk 0 computed up front; chunk j+1 interleaved into block j
            xt_cur = load_x(0)
            for sl in range(4):
                qkv_slice(0, xt_cur, sl)
            for j in range(TJ):
                if j < TJ - 1:
                    xt_nxt = load_x(j + 1)
                yts = []
                for p in range(4):
                    pair = (2 * p, 2 * p + 1)
                    yt = ysbp.tile([128, 512], F32R, name="yt", tag="yt")
                    yts.append(yt)
                    att = {}
                    for kcg in range(2 * (j + 1)):
                        sp = {h: sps.tile([128, 1024], F32, name="sp", tag="sp")
                              for h in pair}
                        for u in range(2):
                            kc = 2 * kcg + u
                            d = max(0, (kc - 4 * j) * 128)
                            for h in pair:
                                base = (h % 2) * 64
                                ksl = qkT[4 + h // 2][base:base + 64,
                                                     kc * 128:(kc + 1) * 128]
                                qsl = qkT[h // 2][base:base + 64,
                                                  j * 512 + d:(j + 1) * 512]
                                nc.tensor.matmul(
                                    sp[h][:, u * 512 + d:(u + 1) * 512],
                                    (ksl), (qsl),
                                    start=True, stop=True,
                                    skip_group_check=True)
                        # exp regions skip the never-written diagonal holes;
                        # the diag blocks get the causal mask applied after
                        # via a DVE multiply with a 0/1 triangular tile.
                        if kcg == 2 * j:
                            regs = ((0, 512), (640, 1024))
                            diag = (0, 640)
                        elif kcg == 2 * j + 1:
                            regs = ((256, 512), (896, 1024))
                            diag = (256, 896)
                        else:
                            regs = ((0, 1024),)
                            diag = None
                        for h in pair:
                            if j == 0:
                                at = ap0.tile([128, 1024], F32R, name="at0",
                                              tag="at0")
                                for (a, b) in regs:
                                    nc.scalar.activation(
                                        at[:, a:b], sp[h][:, a:b], EXP,
                                        scale=SCALE)
                                tmask = trimr
                            else:
                                at = ap8.tile([128, 1024], F8, name="at8",
                                              tag="at8")
                                for (a, b) in regs:
                                    nc.scalar.activation(
                                        at[:, a:b], sp[h][:, a:b], EXP,
                                        scale=SCALE, bias=bias_sb[:])
                                tmask = trim8
                            if diag is not None:
                                for dd in diag:
                                    nc.vector.tensor_mul(
                                        at[:, dd:dd + 128],
                                        at[:, dd:dd + 128], tmask[:])
                            att[(h, kcg)] = at
                    # PV + combine. odd head's y lands on partitions 64:128
                    # (vdr layout) so no remap DMA is needed for j>=1.
                    for h in (pair[1], pair[0]):
                        yp = yps.tile([128, 512], F32, name="yp", tag="yp")
                        if j == 0:
                            for kcg in range(2):
                                for u in range(2):
                                    kc = 2 * kcg + u
                                    d = kc * 128
                                    nc.tensor.matmul(
                                        yp[0:65, d:512],
                                        (vaug0[kc][:, h * 65:h * 65 + 65]),
                                        (att[(h, kcg)][:, u * 512 + d:(u + 1) * 512]),
                                        start=(kc == 0), stop=(kc == 3),
                                        skip_group_check=True)
                        else:
                            for kcg in range(2 * (j + 1)):
                                at3 = att[(h, kcg)].rearrange(
                                    "p (ko q) -> p ko q", q=512)
                                kc0, kc1 = 2 * kcg, 2 * kcg + 1
                                d0 = max(0, (kc0 - 4 * j) * 128)
                                d1 = max(0, (kc1 - 4 * j) * 128)
                                if d1 > d0:
                                    nc.tensor.matmul(
                                        yp[:, d0:d1],
                                        vdr[kcg][:, 0, h, :],
                                        at3[:, 0, d0:d1],
                                        start=False, stop=False,
                                        skip_group_check=True)
                                nc.tensor.matmul(
                                    yp[:, d1:512],
                                    vdr[kcg][:, :, h, :],
                                    at3[:, :, d1:512],
                                    start=(kcg == 0), stop=(kcg == 2 * j + 1),
                                    perf_mode=DR, skip_group_check=True)
                        # ---- combine + normalize (no ACT-queue DMAs).
                        # partition_broadcast on HW needs src AND dst at
                        # partition 0, so dn is routed through p0; odd heads
                        # get that free from their vdr layout.
                        rc = cmb.tile([128, 512], F32, name="rc", tag="rc")
                        bp = cmb.tile([128, 512], F32, name="bp", tag="bp")
                        if j == 0:
                            # y8 @ 0:64, dn @ 64 for both parities
                            dnc = cmb.tile([128, 512], F32, name="dnc",
                                           tag="sc0")
                            dn0 = cmb.tile([1, 512], F32, name="dn0",
                                           tag="dn0")
                            nc.vector.tensor_copy(dnc[64:65, :], yp[64:65, :])
                            nc.sync.dma_start(dn0[0:1, :], dnc[64:65, :])
                            nc.vector.reciprocal(rc[0:1, :], dn0[0:1, :])
                            nc.gpsimd.partition_broadcast(bp[0:64, :],
                                                          rc[0:1, :])
                            if h % 2 == 0:
                                nc.vector.tensor_mul(yt[0:64, :], yp[0:64, :],
                                                     bp[0:64, :])
                            else:
                                ytmp = cmb.tile([64, 512], F32R, name="ytmp",
                                                tag="ytmp")
                                nc.vector.tensor_mul(ytmp[:], yp[0:64, :],
                                                     bp[0:64, :])
                                nc.sync.dma_start(yt[64:128, :], ytmp[:])
                        else:
                            sc = cmb.tile([128, 512], F32, name="sc", tag="sc0")
                            tsum = cmb.tile([128, 512], F32, name="tsum",
                                            tag="tsum")
                            # full-width copy frees yp after one DVE op; the
                            # corr remap + tsum then read sc, not PSUM.
                            nc.vector.tensor_copy(sc[:, :], yp[:, :])
                            dn0 = cmb.tile([1, 512], F32, name="dn0",
                                           tag="dn0")
                            if h % 2 == 0:
                                # y8@0:64, resid@64:127, dn@127
                                nc.sync.dma_start(corr_sb[0:63, :],
                                                  sc[64:127, :])
                                nc.sync.dma_start(dn0[0:1, :], sc[127:128, :])
                                nc.vector.reciprocal(rc[0:1, :], dn0[0:1, :])
                                nc.gpsimd.partition_broadcast(bp[0:64, :],
                                                              rc[0:1, :])
                                # corr_sb[63] is memset 0, so a plain add
                                # applies the residual correction
                                nc.vector.tensor_add(
                                    tsum[0:64, :], corr_sb[0:64, :],
                                    sc[0:64, :])
                                nc.vector.tensor_mul(yt[0:64, :], tsum[0:64, :],
                                                     bp[0:64, :])
                            else:
                                # resid@0:63, dn@63, y8@64:128
                                nc.sync.dma_start(corr_sb[64:127, :],
                                                  sc[0:63, :])
                                nc.sync.dma_start(dn0[0:1, :], sc[63:64, :])
                                nc.vector.reciprocal(rc[0:1, :], dn0[0:1, :])
                                nc.gpsimd.partition_broadcast(bp[0:128, :],
                                                              rc[0:1, :])
                                nc.vector.tensor_add(
                                    tsum[64:128, :], corr_sb[64:128, :],
                                    sc[64:128, :])
                                nc.vector.tensor_mul(yt[64:128, :],
                                                     tsum[64:128, :],
                                                     bp[64:128, :])
                    # interleaved qkv slice for the next token chunk
                    if j < TJ - 1:
                        qkv_slice(j + 1, xt_nxt, p)
                # ---- proj for this token block ----
                for ts in range(4):
                    ot = osbp.tile([128, 1024], F32, name="ot", tag="ot")
                    for co in range(2):
                        ps = yps.tile([128, 512], F32, name="pps", tag="yp")
                        for pp in range(4):
                            nc.tensor.matmul(
                                ps[:],
                                (yts[pp][:, ts * 128:(ts + 1) * 128]),
                                (wp_sb[pp][:, co * 512:(co + 1) * 512]),
                                start=(pp == 0), stop=(pp == 3))
                        nc.vector.tensor_copy(ot[:, co * 512:(co + 1) * 512], ps[:])
                    nc.sync.dma_start(
                        yout[(j * 4 + ts) * 128:(j * 4 + ts + 1) * 128, :], ot[:])

    nc.compile()
    return nc


# ---------------- host-side sharding ----------------

def shard_inputs(x, w_qkv, w_proj):
    """Full inputs -> list of 8 per-core input maps."""
    import ml_dtypes
    r = np.arange(128)
    trim = (r[None, :] >= r[:, None]).astype(np.float32)  # 1 where q >= k
    in_maps = []
    for core in range(8):
        b, g = core // 2, core % 2
        sl = slice(g * CL, (g + 1) * CL)
        in_maps.append(dict(
            xT=np.ascontiguousarray(x[b].T).astype(ml_dtypes.bfloat16),
            wqk=np.ascontiguousarray(
                np.concatenate([w_qkv[:, sl], w_qkv[:, C + g * CL:C + (g + 1) * CL]],
                               axis=1)).astype(ml_dtypes.bfloat16),
            wv=np.ascontiguousarray(w_qkv[:, 2 * C + g * CL:2 * C + (g + 1) * CL]).astype(ml_dtypes.bfloat16),
            wp=np.ascontiguousarray(w_proj[sl, :].reshape(4, 128, C)),
            trim=trim,
        ))
    return in_maps


def unshard_output(results, b_proj):
    """Per-core partial [T, C] projections -> full [B, T, C] output."""
    out = np.empty((4, T, C), dtype=np.float32)
    for b in range(4):
        out[b] = results[2 * b]["yout"] + results[2 * b + 1]["yout"]
    out += b_proj[None, None, :]
    return out


_CACHE = {}


def kernel(x, w_qkv, w_proj, b_proj):
    from concourse.bass_utils import run_bass_kernel_spmd
    if "nc" not in _CACHE:
        _CACHE["nc"] = build_nc()
    nc = _CACHE["nc"]
    in_maps = shard_inputs(np.asarray(x, np.float32),
                           np.asarray(w_qkv, np.float32),
                           np.asarray(w_proj, np.float32))
    res = run_bass_kernel_spmd(nc, in_maps, core_ids=list(range(8)))
    return unshard_output(res.results, np.asarray(b_proj, np.float32))


# revision 21
# speedup vs baseline: 1.1021x; 1.0022x over previous
"""v5: ACT-queue unblocking. Causal self-attention (B=4, T=2048, C=1024,
H=16) on 8 TRN2 NeuronCores.

Sharding (tensor-parallel over batch x head-group): core c handles batch c//2
and heads [8*(c%2), 8*(c%2)+8). Host sums the two head-group partials per
batch and adds the bias.

v5 on top of v4: the sim trace showed ScalarE (exp) is the bottleneck engine
and the combine DMAs dispatched from its queue stall it ~2.4us per head
while they wait for their source. Changes:
  - dn DMAs eliminated: reciprocal runs directly on the denominator
    partition (p64 for j=0, p127/p63 for j>=1) and partition_broadcast
    sources that partition; nothing is remapped to partition 0.
  - odd heads' vdr layout is [resid 63 | 1 | v8 64] so their PV output
    lands on partitions 64:128 directly -> no ytmp remap DMA for j>=1.
  - remaining remap DMAs (corr, j=0 odd ytmp) moved to the gpsimd
    (SWDGE) queue; the ACT queue carries no per-iteration DMAs at all.
  - startup weight/const DMAs moved to the ACT queue so the SP queue
    starts the x chunk-0 loads immediately.
"""
from contextlib import ExitStack

import numpy as np
import concourse.bass as bass
import concourse.mybir as mybir
import concourse.tile as tile
from concourse import bacc

F32 = mybir.dt.float32
F32R = mybir.dt.float32r
BF16 = mybir.dt.bfloat16
F8 = mybir.dt.float8e4
EXP = mybir.ActivationFunctionType.Exp
DR = mybir.MatmulPerfMode.DoubleRow
MUL = mybir.AluOpType.mult
ADD = mybir.AluOpType.add

T = 2048          # tokens
C = 1024          # channels
NH = 8            # local heads
HD = 64           # head dim
CL = NH * HD      # local channels (512)
TJ = T // 512     # 4 q-chunks of 512
KC = T // 128     # 16 k-chunks of 128
SCALE = HD ** -0.5
BIAS = -3.0       # exp bias for fp8 att storage (cancels in normalization)


def build_nc(loop_reps: int | None = None):
    nc = bacc.Bacc("TRN2", target_bir_lowering=False, debug=False)
    xT = nc.declare_dram_parameter("xT", [C, T], BF16, isOutput=False)
    wqk = nc.declare_dram_parameter("wqk", [C, 2 * CL], BF16, isOutput=False)
    wv = nc.declare_dram_parameter("wv", [C, CL], BF16, isOutput=False)
    wp = nc.declare_dram_parameter("wp", [4, 128, C], F32R, isOutput=False)
    idn = nc.declare_dram_parameter("idn", [128, 128], BF16, isOutput=False)
    maskm = nc.declare_dram_parameter("maskm", [128, 128], BF16, isOutput=False)
    onec = nc.declare_dram_parameter("onec", [128, 64], F32R, isOutput=False)
    one8 = nc.declare_dram_parameter("one8", [128, 8], F8, isOutput=False)
    mskc = nc.declare_dram_parameter("mskc", [128, 1], F32, isOutput=False)
    yout = nc.declare_dram_parameter("yout", [T, C], F32, isOutput=True)

    with ExitStack() as ctx:
        ctx.enter_context(nc.allow_low_precision(
            reason="fp8 PV with residual correction; bf16 scores; fp32r elsewhere"))
        tc = ctx.enter_context(tile.TileContext(nc, pool_alloc_mode="queue"))

        # ---- persistent pools ----
        # weight loads go on the ACT queue (idle until the first exp) so the
        # SP queue reaches the per-rep x loads immediately.
        w1 = ctx.enter_context(tc.tile_pool(name="w1", bufs=1))
        wqk_sb = [w1.tile([128, 2 * CL], BF16, name=f"wqk{ci}") for ci in range(8)]
        wv_sb = [w1.tile([128, CL], BF16, name=f"wv{ci}") for ci in range(8)]
        for ci in range(8):
            nc.scalar.dma_start(wqk_sb[ci][:], wqk[ci * 128:(ci + 1) * 128, :])
        for ci in range(8):
            nc.scalar.dma_start(wv_sb[ci][:], wv[ci * 128:(ci + 1) * 128, :])

        consts = ctx.enter_context(tc.tile_pool(name="consts", bufs=1))
        idn_sb = consts.tile([128, 128], BF16)
        maskm_sb = consts.tile([128, 128], BF16)
        msk_sb = consts.tile([128, 1], F32)
        bias_sb = consts.tile([128, 1], F32)
        nc.scalar.dma_start(idn_sb[:], idn[:])
        nc.scalar.dma_start(maskm_sb[:], maskm[:])
        nc.scalar.dma_start(msk_sb[:], mskc[:])
        nc.gpsimd.memset(bias_sb[:], BIAS)

        qk_pool = ctx.enter_context(tc.tile_pool(name="qk_pool", bufs=1))
        qkT = [qk_pool.tile([128, T], BF16, name=f"qkT{fi}") for fi in range(8)]
        v0_pool = ctx.enter_context(tc.tile_pool(name="v0_pool", bufs=1))
        vaug0 = [v0_pool.tile([128, NH * 65], F32R, name=f"vaug0{tt}")
                 for tt in range(4)]
        vdr_pool = ctx.enter_context(tc.tile_pool(name="vdr_pool", bufs=1))
        vdr = [vdr_pool.tile([128, 2, NH, 128], F8, name=f"vdr{g}")
               for g in range(KC // 2)]
        wp_pool = ctx.enter_context(tc.tile_pool(name="wp_pool", bufs=1))
        wp_sb = [wp_pool.tile([128, C], F32R, name=f"wp{pp}") for pp in range(4)]
        for pp in range(4):
            nc.scalar.dma_start(wp_sb[pp][:], wp[pp, :, :])
        # persistent corr staging tile; rows 63/127 are never written by the
        # remap DMAs but are read (x msk==0), so zero them once.
        corr_pool = ctx.enter_context(tc.tile_pool(name="corr_pool", bufs=1))
        corr_sb = corr_pool.tile([128, 512], F32, name="corr_sb")
        nc.gpsimd.memset(corr_sb[32:64, :], 0.0)
        nc.gpsimd.memset(corr_sb[96:128, :], 0.0)
        # ones slots are never overwritten by the per-rep v writes: init once.
        # even heads use [v8 64 | resid 63 | 1@127] (y8@0:64, dn@p127);
        # odd heads use [resid 63 | 1@63 | v8 64] (dn@p63, y8@p64:128) so PV
        # puts their y on partitions 64:128 directly. All fp8 DVE writes
        # stay at byte offsets 0/64 within the head segment.
        # DVE writes (same queue as the per-rep v writes, byte-granular)
        # instead of 1-byte DMAs: the DMA path does sub-word
        # read-modify-write on SBUF which races with the first rep's DVE
        # writes to neighboring bytes of the same word.
        ones32 = consts.tile([128, 8], F32)
        nc.gpsimd.memset(ones32[:], 1.0)
        for g in range(KC // 2):
            vg4 = vdr[g].rearrange("p ko (pr par) c -> p ko pr par c", par=2)
            for ko in range(2):
                nc.vector.tensor_copy(vg4[:, ko, :, 0, 127:128], ones32[:, 0:4])
                nc.vector.tensor_copy(vg4[:, ko, :, 1, 63:64], ones32[:, 4:8])
        for tt in range(4):
            va = vaug0[tt].rearrange("p (h s) -> p h s", s=65)
            nc.vector.tensor_copy(va[:, :, 64:65], ones32[:, 0:8])

        loop = tc.For_i(0, loop_reps) if loop_reps is not None else None
        if loop is not None:
            ctx.enter_context(loop)

        with tc.tile_pool(name="xp", bufs=16) as xp, \
             tc.tile_pool(name="ps1", bufs=2, space="PSUM") as ps1, \
             tc.tile_pool(name="ap0", bufs=2) as ap0, \
             tc.tile_pool(name="ap8", bufs=6) as ap8, \
             tc.tile_pool(name="ysbp", bufs=6) as ysbp, \
             tc.tile_pool(name="cmb", bufs=3) as cmb, \
             tc.tile_pool(name="osb", bufs=2) as osbp, \
             tc.tile_pool(name="sps", bufs=2, space="PSUM") as sps, \
             tc.tile_pool(name="yps", bufs=2, space="PSUM") as yps:
            def load_x(tj):
                xt = []
                for ci in range(8):
                    t_ = xp.tile([128, 512], BF16, name="xt", tag="xt")
                    nc.sync.dma_start(t_[:], xT[ci * 128:(ci + 1) * 128,
                                                 tj * 512:(tj + 1) * 512])
                    xt.append(t_)
                return xt

            def qkv_slice(tj, xt, sl):
                """Emit qkv work slice sl (0..3) for token chunk tj.

                Slice sl produces q-tile sl and k-tile 4+sl so that pair
                sl's first QK block is ready after slice sl alone (the old
                (2sl, 2sl+1) mapping delivered k tiles only in slices 2-3,
                serializing the whole chunk-0 projection before any
                attention could start)."""
                for fi in (sl, 4 + sl):
                    ps = ps1.tile([128, 512], F32, name="qkps", tag="qkps")
                    for ci in range(8):
                        nc.tensor.matmul(
                            ps[:],
                            (wqk_sb[ci][:, fi * 128:(fi + 1) * 128]),
                            (xt[ci][:]),
                            start=(ci == 0), stop=(ci == 7))
                    nc.vector.tensor_copy(qkT[fi][:, tj * 512:(tj + 1) * 512], ps[:])
                ts = sl
                tt = tj * 4 + ts
                ps = ps1.tile([128, 512], F32, name="vps", tag="qkps")
                for ci in range(8):
                    nc.tensor.matmul(
                        ps[:],
                        (xt[ci][:, ts * 128:(ts + 1) * 128]),
                        (wv_sb[ci][:]),
                        start=(ci == 0), stop=(ci == 7))
                ps4 = ps.rearrange("p (pr par s) -> p pr par s", par=2, s=64)
                g, ko = tt // 2, tt % 2
                vg4 = vdr[g].rearrange("p ko (pr par) c -> p ko pr par c", par=2)
                # even heads: [v8 | resid | 1@127]
                nc.vector.tensor_copy(vg4[:, ko, :, 0, 0:64], ps4[:, :, 0, :])
                nc.vector.tensor_sub(vg4[:, ko, :, 0, 64:127],
                                     ps4[:, :, 0, 0:63], vg4[:, ko, :, 0, 0:63])
                # odd heads: [resid | 1@63 | v8]
                nc.vector.tensor_copy(vg4[:, ko, :, 1, 64:128], ps4[:, :, 1, :])
                nc.vector.tensor_sub(vg4[:, ko, :, 1, 0:63],
                                     ps4[:, :, 1, 0:63], vg4[:, ko, :, 1, 64:127])
                if tt < 4:
                    va = vaug0[tt].rearrange("p (h s) -> p h s", s=65)
                    nc.vector.tensor_copy(va[:, :, 0:64],
                                          ps.rearrange("p (h s) -> p h s", s=64)[:])

            # chunk 0 computed up front; chunk j+1 interleaved into block j
            xt_cur = load_x(0)
            for sl in range(4):
                qkv_slice(0, xt_cur, sl)
            for j in range(TJ):
                if j < TJ - 1:
                    xt_nxt = load_x(j + 1)
                yts = []
                for p in range(4):
                    pair = (2 * p, 2 * p + 1)
                    yt = ysbp.tile([128, 512], F32R, name="yt", tag="yt")
                    yts.append(yt)
                    att = {}
                    for kcg in range(2 * (j + 1)):
                        sp = {h: sps.tile([128, 1024], F32, name="sp", tag="sp")
                              for h in pair}
                        for u in range(2):
                            kc = 2 * kcg + u
                            d = max(0, (kc - 4 * j) * 128)
                            for h in pair:
                                base = (h % 2) * 64
                                ksl = qkT[4 + h // 2][base:base + 64,
                                                     kc * 128:(kc + 1) * 128]
                                qsl = qkT[h // 2][base:base + 64,
                                                  j * 512 + d:(j + 1) * 512]
                                nc.tensor.matmul(
                                    sp[h][:, u * 512 + d:(u + 1) * 512],
                                    (ksl), (qsl),
                                    start=True, stop=(kc < 4 * j),
                                    skip_group_check=True)
                            if kc >= 4 * j:
                                for h in pair:
                                    nc.tensor.matmul(
                                        sp[h][:, u * 512 + d:u * 512 + d + 128],
                                        idn_sb[:], maskm_sb[:],
                                        start=False, stop=True,
                                        skip_group_check=True)
                        # exp regions skip the never-written diagonal holes
                        # (keeps the PSUM reads fully initialized)
                        if kcg == 2 * j:
                            regs = ((0, 512), (640, 1024))
                        elif kcg == 2 * j + 1:
                            regs = ((256, 512), (896, 1024))
                        else:
                            regs = ((0, 1024),)
                        for h in pair:
                            if j == 0:
                                at = ap0.tile([128, 1024], F32R, name="at0",
                                              tag="at0")
                                for (a, b) in regs:
                                    nc.scalar.activation(
                                        at[:, a:b], sp[h][:, a:b], EXP,
                                        scale=SCALE)
                            else:
                                at = ap8.tile([128, 1024], F8, name="at8",
                                              tag="at8")
                                for (a, b) in regs:
                                    nc.scalar.activation(
                                        at[:, a:b], sp[h][:, a:b], EXP,
                                        scale=SCALE, bias=bias_sb[:])
                            att[(h, kcg)] = at
                    # PV + combine. odd head's y lands on partitions 64:128
                    # (vdr layout) so no remap DMA is needed for j>=1.
                    for h in (pair[1], pair[0]):
                        yp = yps.tile([128, 512], F32, name="yp", tag="yp")
                        if j == 0:
                            for kcg in range(2):
                                for u in range(2):
                                    kc = 2 * kcg + u
                                    d = kc * 128
                                    nc.tensor.matmul(
                                        yp[0:65, d:512],
                                        (vaug0[kc][:, h * 65:h * 65 + 65]),
                                        (att[(h, kcg)][:, u * 512 + d:(u + 1) * 512]),
                                        start=(kc == 0), stop=(kc == 3),
                                        skip_group_check=True)
                        else:
                            for kcg in range(2 * (j + 1)):
                                at3 = att[(h, kcg)].rearrange(
                                    "p (ko q) -> p ko q", q=512)
                                kc0, kc1 = 2 * kcg, 2 * kcg + 1
                                d0 = max(0, (kc0 - 4 * j) * 128)
                                d1 = max(0, (kc1 - 4 * j) * 128)
                                if d1 > d0:
                                    nc.tensor.matmul(
                                        yp[:, d0:d1],
                                        vdr[kcg][:, 0, h, :],
                                        at3[:, 0, d0:d1],
                                        start=False, stop=False,
                                        skip_group_check=True)
                                nc.tensor.matmul(
                                    yp[:, d1:512],
                                    vdr[kcg][:, :, h, :],
                                    at3[:, :, d1:512],
                                    start=(kcg == 0), stop=(kcg == 2 * j + 1),
                                    perf_mode=DR, skip_group_check=True)
                        # ---- combine + normalize (no ACT-queue DMAs).
                        # partition_broadcast on HW needs src AND dst at
                        # partition 0, so dn is routed through p0; odd heads
                        # get that free from their vdr layout.
                        rc = cmb.tile([128, 512], F32, name="rc", tag="rc")
                        bp = cmb.tile([128, 512], F32, name="bp", tag="bp")
                        if j == 0:
                            # y8 @ 0:64, dn @ 64 for both parities
                            dnc = cmb.tile([128, 512], F32, name="dnc",
                                           tag="sc0")
                            dn0 = cmb.tile([1, 512], F32, name="dn0",
                                           tag="dn0")
                            nc.vector.tensor_copy(dnc[64:65, :], yp[64:65, :])
                            nc.sync.dma_start(dn0[0:1, :], dnc[64:65, :])
                            nc.vector.reciprocal(rc[0:1, :], dn0[0:1, :])
                            nc.gpsimd.partition_broadcast(bp[0:64, :],
                                                          rc[0:1, :])
                            if h % 2 == 0:
                                nc.vector.tensor_mul(yt[0:64, :], yp[0:64, :],
                                                     bp[0:64, :])
                            else:
                                ytmp = cmb.tile([64, 512], F32R, name="ytmp",
                                                tag="ytmp")
                                nc.vector.tensor_mul(ytmp[:], yp[0:64, :],
                                                     bp[0:64, :])
                                nc.sync.dma_start(yt[64:128, :], ytmp[:])
                        else:
                            sc = cmb.tile([128, 512], F32, name="sc", tag="sc0")
                            tsum = cmb.tile([128, 512], F32, name="tsum",
                                            tag="tsum")
                            # full-width copy frees yp after one DVE op; the
                            # corr remap + tsum then read sc, not PSUM.
                            nc.vector.tensor_copy(sc[:, :], yp[:, :])
                            dn0 = cmb.tile([1, 512], F32, name="dn0",
                                           tag="dn0")
                            if h % 2 == 0:
                                # y8@0:64, resid@64:127, dn@127
                                nc.sync.dma_start(corr_sb[0:63, :],
                                                  sc[64:127, :])
                                nc.sync.dma_start(dn0[0:1, :], sc[127:128, :])
                                nc.vector.reciprocal(rc[0:1, :], dn0[0:1, :])
                                nc.gpsimd.partition_broadcast(bp[0:64, :],
                                                              rc[0:1, :])
                                # corr_sb[63] is memset 0, so a plain add
                                # applies the residual correction
                                nc.vector.tensor_add(
                                    tsum[0:64, :], corr_sb[0:64, :],
                                    sc[0:64, :])
                                nc.vector.tensor_mul(yt[0:64, :], tsum[0:64, :],
                                                     bp[0:64, :])
                            else:
                                # resid@0:63, dn@63, y8@64:128
                                nc.sync.dma_start(corr_sb[64:127, :],
                                                  sc[0:63, :])
                                nc.sync.dma_start(dn0[0:1, :], sc[63:64, :])
                                nc.vector.reciprocal(rc[0:1, :], dn0[0:1, :])
                                nc.gpsimd.partition_broadcast(bp[0:128, :],
                                                              rc[0:1, :])
                                nc.vector.tensor_add(
                                    tsum[64:128, :], corr_sb[64:128, :],
                                    sc[64:128, :])
                                nc.vector.tensor_mul(yt[64:128, :],
                                                     tsum[64:128, :],
                                                     bp[64:128, :])
                    # interleaved qkv slice for the next token chunk
                    if j < TJ - 1:
                        qkv_slice(j + 1, xt_nxt, p)
                # ---- proj for this token block ----
                for ts in range(4):
                    ot = osbp.tile([128, 1024], F32, name="ot", tag="ot")
                    for co in range(2):
                        ps = yps.tile([128, 512], F32, name="pps", tag="yp")
                        for pp in range(4):
                            nc.tensor.matmul(
                                ps[:],
                                (yts[pp][:, ts * 128:(ts + 1) * 128]),
                                (wp_sb[pp][:, co * 512:(co + 1) * 512]),
                                start=(pp == 0), stop=(pp == 3))
                        nc.vector.tensor_copy(ot[:, co * 512:(co + 1) * 512], ps[:])
                    nc.sync.dma_start(
                        yout[(j * 4 + ts) * 128:(j * 4 + ts + 1) * 128, :], ot[:])

    nc.compile()
    return nc


# ---------------- host-side sharding ----------------

def shard_inputs(x, w_qkv, w_proj):
    """Full inputs -> list of 8 per-core input maps."""
    import ml_dtypes
    idn = np.eye(128, dtype=ml_dtypes.bfloat16)
    r = np.arange(128)
    maskm = np.where(r[:, None] > r[None, :], -1e9, 0.0).astype(ml_dtypes.bfloat16)
    one8 = np.ones((128, 8), ml_dtypes.float8_e4m3)
    msk = np.ones((128, 1), np.float32)
    msk[63] = 0.0
    msk[127] = 0.0
    in_maps = []
    for core in range(8):
        b, g = core // 2, core % 2
        sl = slice(g * CL, (g + 1) * CL)
        in_maps.append(dict(
            xT=np.ascontiguousarray(x[b].T).astype(ml_dtypes.bfloat16),
            wqk=np.ascontiguousarray(
                np.concatenate([w_qkv[:, sl], w_qkv[:, C + g * CL:C + (g + 1) * CL]],
                               axis=1)).astype(ml_dtypes.bfloat16),
            wv=np.ascontiguousarray(w_qkv[:, 2 * C + g * CL:2 * C + (g + 1) * CL]).astype(ml_dtypes.bfloat16),
            wp=np.ascontiguousarray(w_proj[sl, :].reshape(4, 128, C)),
            idn=idn, maskm=maskm, onec=np.ones((128, 64), np.float32),
            one8=one8, mskc=msk,
        ))
    return in_maps


def unshard_output(results, b_proj):
    """Per-core partial [T, C] projections -> full [B, T, C] output."""
    out = np.empty((4, T, C), dtype=np.float32)
    for b in range(4):
        out[b] = results[2 * b]["yout"] + results[2 * b + 1]["yout"]
    out += b_proj[None, None, :]
    return out


_CACHE = {}


def kernel(x, w_qkv, w_proj, b_proj):
    from concourse.bass_utils import run_bass_kernel_spmd
    if "nc" not in _CACHE:
        _CACHE["nc"] = build_nc()
    nc = _CACHE["nc"]
    in_maps = shard_inputs(np.asarray(x, np.float32),
                           np.asarray(w_qkv, np.float32),
                           np.asarray(w_proj, np.float32))
    res = run_bass_kernel_spmd(nc, in_maps, core_ids=list(range(8)))
    return unshard_output(res.results, np.asarray(b_proj, np.float32))
